# revision 1
# baseline (speedup 1.0000x reference)
"""DualMem retrieval kernel for Trainium2 (8 NeuronCores, Bass/Tile).

Math (per reference):
    sim[b,c,m]  = <img[b], mem[c,m]>
    w           = exp(-beta * (1 - sim))
    adapt[b,c]  = sum_m mem[c,m] * w[b,c,m]
    logits[b,c] = 100 * <img[b], adapt[b,c] / ||adapt[b,c]||>

Key algebraic reduction (avoids materializing adapt [B,C,D]):
    numer[b,c]  = <img[b], adapt[b,c]> = sum_m w[b,c,m] * sim[b,c,m]
    denom[b,c]  = ||adapt[b,c]||^2     = w^T G_c w,  G_c = mem_c @ mem_c^T  (11x11 Gram)
    logits      = 100 * numer / sqrt(denom)

Sharding: classes C=1000 split 125 per core across 8 cores (mem bank fully
sharded; only img replicated).

Per-core layout: groups of 11 classes x 11 memories = 121 partitions (pad to
128); 12 groups cover 132 >= 125 class slots.  The bf16 memory bank arrives
class-major and is xbar-DMA-transposed on-chip to [d, cm]; img, the Gram
mask, and the 0/1 class-sum matrix ride the same transpose stream (all
bf16-exact), so there are no plain input DMAs at all.  Groups are processed
in blocks of [4,4,3,1] sharing PSUM banks (per-element has_written makes
disjoint column ranges in one bank legal accumulation groups; the bank-level
software check is skipped):
    su bank [128, gn*128]: per group k, cols 128k+0:64  = sim (acc over d)
                                        cols 128k+64:128 = u = G_masked^T w
    G bank  [128, gn*128]: per group k, cols 128k:128k+128 = Gram (acc over d)
Downstream per block: one batched exp, one masked-Gram copy (the mandatory
PSUM->SBUF move), gn u-matmuls, one broadcast mul building [w*sim | w*u],
one 0/1 "E" matmul summing over m per class -> [numer | denom] in PSUM.
Finals read PSUM directly and use 100/sqrt(d) = exp(-0.5*ln(d) + ln(100));
Ln and Exp share one ACT function table, so the table is loaded exactly
once.  The small trailing blocks keep the end-of-kernel dependency chain
short, per-block finals overlap earlier compute, each block's sim/G matmuls
are emitted ahead of the previous block's downstream so the PE queue favors
them, and dependency-free junk matmuls warm the PE (HAM clock gate) during
the transpose startup window.
"""

import sys

sys.path.insert(0, "/opt/trn_rl_repo")

import ml_dtypes
import numpy as np

B, C, M, D = 64, 1000, 11, 1024
BETA = 5.5
N_CORES = 8
C_PER = C // N_CORES          # 125 classes per core
CPG = 11                      # classes per group
NG = 12                       # groups per core (132 class slots >= 125)
PG = CPG * M                  # 121 used partitions per group
DCH = D // 128                # 8 d-chunks
ROWS = NG * 128               # 1536 class-major rows per core

_cache = {}


def _build():
    import concourse.mybir as mybir
    import concourse.tile as tile
    from concourse import bacc

    # Pin every activation to the one ACT table that holds BOTH Exp and Ln
    # (indices must be preserved — empty the other sets instead of dropping
    # them) so the function table is loaded once and never swapped.
    if not getattr(bacc, "_act_tables_pinned", False):
        real = bacc.get_activation_tables

        def pinned(arch):
            return {k: (v if k == "natural_log_exp_and_others" else set())
                    for k, v in real(arch).items()}
        bacc.get_activation_tables = pinned
        bacc._act_tables_pinned = True

    f32 = mybir.dt.float32
    bf16 = mybir.dt.bfloat16

    nc = bacc.Bacc("TRN2", target_bir_lowering=False, debug=False,
                   num_devices=N_CORES)

    # membf rows: [64 img | 128 mask/em | 1536 class-major memory rows].
    # The xbar transpose of the leading rows lands imgT, the block-diagonal
    # Gram mask, and the 0/1 class-sum matrix (all bf16-exact) in exactly
    # the layouts the compute wants — no separate const loads at all.
    EXT = 192
    membf = nc.dram_tensor("membf", [EXT + 11 * 128 + 48, D], bf16,
                           kind="ExternalInput")
    out = nc.dram_tensor("out", [16, NG * 64], f32, kind="ExternalOutput")

    with tile.TileContext(nc) as tc:
        with (
            tc.tile_pool(name="const", bufs=1) as const,
            tc.tile_pool(name="sb", bufs=3) as sb,
            tc.tile_pool(name="ps_su", bufs=2, space="PSUM") as ps_su,
            tc.tile_pool(name="ps_g", bufs=2, space="PSUM") as ps_g,
            tc.tile_pool(name="ps_nd", bufs=1, space="PSUM") as ps_nd,
        ):
            # memT[d % 128, d_chunk, cm]; transpose batches sized so compute
            # can start right after img+g0 land:
            #   b0: img(64)+g0(128)  b1: mask/em(128)  b2: g1  b3: g2,g3
            #   b4..b7: g4..g11 two groups each
            bat_rows = [192, 128, 128, 256, 256, 128, 128, 128, 128, 128, 48]
            mt = [const.tile([128, 5 if q == 1 else DCH, r], bf16,
                             name=f"mt{q}", tag=f"mt{q}")
                  for q, r in enumerate(bat_rows)]
            # batch 0 is transposed as two column-halves into separate
            # tiles so g0's first d-chunks are compute-ready after half
            # the data
            mt0 = [const.tile([128, 4, 192], bf16, name=f"mt0{h}",
                              tag=f"mt0{h}") for h in range(2)]
            # group g -> (batch tile, col offset); g0 handled via mt0
            gloc = {0: (None, 64), 1: (mt[2], 0), 2: (mt[3], 0),
                    3: (mt[3], 128)}
            for g in range(4, 6):
                gloc[g] = (mt[4 + (g - 4) // 2], 128 * ((g - 4) % 2))
            gloc[5] = (mt[4], 128)
            for g in range(6, NG):
                gloc[g] = (mt[5 + (g - 6)], 0)

            def it_chunk(i):
                return mt0[i // 4][:, i % 4, 0:64]

            def blk_chunk(g, i, off, gw):
                if g == 0:
                    return mt0[i // 4][:, i % 4, 64:64 + gw]
                tile_, o = gloc[g]
                return tile_[:, i, o + (off - o):o + (off - o) + gw]
            mask_bf = mt[1][:, 0:4, :]               # [128, 4, 128] bf16
            em_bf = mt[1][:, 4, 0:16]                # [128, 16] bf16
            lg = const.tile([16, NG * 64], f32)
            bias_exp = const.tile([128, 1], f32)
            bias_eps = const.tile([16, 1], f32)
            bias_ln100 = const.tile([16, 1], f32)
            junk_w = const.tile([128, 16], bf16)
            junk_x = const.tile([128, 512], bf16)
            nc.vector.memset(junk_w[:], 0)
            nc.vector.memset(junk_x[:], 0)
            nc.vector.memset(bias_exp[:], -BETA)
            nc.vector.memset(bias_eps[:], 1e-30)
            nc.vector.memset(bias_ln100[:], float(np.log(100.0)))

            # xbar transposes in issue order; everything (img, mask/em, mem
            # bank) rides the transpose stream — no plain input DMAs at all.
            r0 = 0
            for q, r in enumerate(bat_rows):
                if q == 0:
                    for h in range(2):
                        nc.sync.dma_start(
                            mt0[h][:],
                            membf.ap()[0:192, h * 512:(h + 1) * 512],
                            transpose=True,
                        )
                else:
                    ncol = 5 * 128 if q == 1 else D
                    nc.sync.dma_start(
                        mt[q][:],
                        membf.ap()[r0:r0 + r, 0:ncol],
                        transpose=True,
                    )
                r0 += r

            # blocks of groups sharing PSUM banks: the last blocks are small
            # so the final dependency chain is short and starts early.
            # nd tiles: [numer | denom] per block-set; 2+1+1 PSUM banks.
            BLKS = [(0, 4), (4, 4), (8, 3), (11, 1)]
            nd_a = ps_nd.tile([16, 8 * 128], f32, name="nd_a")
            nd_b = ps_nd.tile([16, 3 * 128], f32, name="nd_b")
            nd_c = ps_nd.tile([16, 1 * 128], f32, name="nd_c")
            nd_dst = [nd_a[:, 0:512], nd_a[:, 512:1024], nd_b[:], nd_c[:]]

            # PE warm-up: the HAM clock gate (and the cost model) halve the
            # PE clock until ~3.4us of sustained activity.  These junk
            # matmuls have no DMA dependency, so they run during the
            # transpose startup window and the real matmuls start warm.
            # They scribble on nd_a, which is rewritten (start=True) later.
            for _ in range(6):
                nc.tensor.matmul(nd_a[:, 0:512], junk_w[:], junk_x[:],
                                 start=True, stop=True,
                                 skip_group_check=True)

            def emit_sims(nb, g0, gn):
                su = ps_su.tile([128, gn * 128], f32, tag="su", name=f"su{nb}")
                gp = ps_g.tile([128, gn * 128], f32, tag="gp", name=f"gp{nb}")
                for k in range(gn):
                    _, off = gloc[g0 + k]
                    gw = 48 if g0 + k == 11 else 128
                    gv = 48 if g0 + k == 11 else PG  # valid Gram columns
                    for i in range(DCH):
                        blk = blk_chunk(g0 + k, i, off, gw)
                        nc.tensor.matmul(su[0:gw, k * 128:k * 128 + 64],
                                         blk, it_chunk(i),
                                         start=(i == 0), stop=(i == DCH - 1),
                                         skip_group_check=True)
                        # i==0 writes all 128 cols so the masked read
                        # later never sees uninitialized PSUM; the 121-127
                        # pad cols keep the i==0 partial and are masked out
                        gvi = gw if i == 0 else gv
                        nc.tensor.matmul(gp[0:gw, k * 128:k * 128 + gvi],
                                         blk, blk[:, 0:gvi],
                                         start=(i == 0), stop=(i == DCH - 1),
                                         skip_group_check=True)
                return su, gp

            def emit_down(nb, gn, su, gp):
                gw = 48 if nb == 3 else 128
                su = su[0:gw]
                gp = gp[0:gw]
                # w = exp(beta*sim - beta) for the whole block at once
                su4 = su.rearrange("p (k t b) -> p k t b", k=gn, t=2)
                w4 = sb.tile([128, gn * 64], bf16, tag="w4",
                             name=f"w4_{nb}")[0:gw]
                w4r = w4.rearrange("p (k b) -> p k b", k=gn)
                nc.scalar.activation(w4r, su4[:, :, 0, :],
                                     mybir.ActivationFunctionType.Exp,
                                     bias=bias_exp[0:gw], scale=BETA)

                # masked Gram -> SBUF (kills cross-class + pad entries)
                gm4 = sb.tile([128, gn * 128], bf16, tag="gm4",
                              name=f"gm4_{nb}")[0:gw]
                if nb == 3:
                    nc.vector.tensor_mul(gm4[:, 0:gw], gp[:, 0:gw],
                                         mask_bf[0:gw, 0, 0:gw])
                else:
                    gp4 = gp.rearrange("p (k j) -> p k j", k=gn)
                    nc.vector.tensor_mul(
                        gm4.rearrange("p (k j) -> p k j", k=gn),
                        gp4, mask_bf[:, 0:gn, :])

                # u_k = G_k^T @ w_k, placed next to sim_k in the same bank
                for k in range(gn):
                    nc.tensor.matmul(su[:, k * 128 + 64:(k + 1) * 128],
                                     gm4[:, k * 128:k * 128 + gw],
                                     w4[:, k * 64:(k + 1) * 64],
                                     start=True, stop=True,
                                     skip_group_check=True)

                # wsq = [w*sim | w*u], one fused mul with w broadcast over t
                wsq = sb.tile([128, gn * 128], bf16, tag="wsq",
                              name=f"wsq_{nb}")[0:gw]
                wq4 = wsq.rearrange("p (k t b) -> p k t b", k=gn, t=2)
                w4b = w4.rearrange("p (k u b) -> p k u b", k=gn, u=1) \
                    .to_broadcast((gw, gn, 2, 64))
                nc.vector.tensor_mul(wq4, su4, w4b)

                # nd[c, :] = [numer | denom] per class for the whole block
                nc.tensor.matmul(nd_dst[nb], em_bf[0:gw], wsq, start=True,
                                 stop=True, skip_group_check=True)

            # Emit each block's sim/G matmuls BEFORE the previous block's
            # downstream ops: ready sim matmuls then outrank earlier blocks'
            # u/nd matmuls in the PE queue, so the last group's sims are not
            # stuck behind them and the closing dependency chain starts
            # sooner.  Pool slot recycling (bufs=2) still paces allocation.
            pend = []
            for nb, (g0, gn) in enumerate(BLKS):
                su, gp = emit_sims(nb, g0, gn)
                pend.append((nb, gn, su, gp))
                if len(pend) >= 2:
                    emit_down(*pend.pop(0))
            for args in pend:
                emit_down(*args)

            # 100/sqrt(denom) = exp(-0.5*ln(denom) + ln(100)) -- Ln and Exp
            # live in the same ACT function table, so no table swap ever
            for half, (nd_t, n, go) in enumerate(
                    [(nd_a, 8, 0), (nd_b, 3, 8), (nd_c, 1, 11)]):
                nd3 = nd_t[:].rearrange("p (g t b) -> p g t b", g=n, t=2)
                s_h = sb.tile([16, n * 64], f32, tag=f"s{half}",
                              name=f"s_{half}")
                nc.scalar.activation(s_h[:], nd3[:, :, 1, :],
                                     mybir.ActivationFunctionType.Ln,
                                     bias=bias_eps[:], scale=1.0)
                r_h = sb.tile([16, n * 64], f32, tag=f"r{half}",
                              name=f"r_{half}")
                nc.scalar.activation(r_h[:], s_h[:],
                                     mybir.ActivationFunctionType.Exp,
                                     bias=bias_ln100[:], scale=-0.5)
                o0 = go * 64
                nc.vector.tensor_mul(lg[:, o0:o0 + n * 64], nd3[:, :, 0, :],
                                     r_h[:])
                if half == 0:
                    nc.sync.dma_start(out.ap()[:, 0:n * 64], lg[:, 0:n * 64])
            nc.sync.dma_start(out.ap()[:, 512:768], lg[:, 512:768])

    nc.compile()
    return nc


def _get_nc():
    if "nc" not in _cache:
        _cache["nc"] = _build()
    return _cache["nc"]


def _prep_inputs(img_features, memorized_image_feat):
    """Host-side formatting: bf16 cast, class padding, group layout."""
    bf = ml_dtypes.bfloat16
    img_b = np.ascontiguousarray(img_features.astype(bf))          # [64, 1024]
    mem_b = memorized_image_feat.astype(bf)                        # [1000,11,1024]

    m1 = np.zeros((128, 128), np.float32)
    for c in range(CPG):
        m1[c * M:(c + 1) * M, c * M:(c + 1) * M] = 1.0
    em = np.zeros((128, 16), np.float32)
    for c in range(CPG):
        em[c * M:(c + 1) * M, c] = 1.0

    # mask/em rows for the transpose stream: transposing maskem[j, 128i+p]
    # yields m1 at d-chunks 0-3 and em^T at chunk 4
    maskem = np.zeros((128, D), bf)
    for i in range(4):
        maskem[:, i * 128:(i + 1) * 128] = m1.T
    maskem[:16, 512:640] = em.T

    in_maps = []
    for k in range(N_CORES):
        sl = mem_b[k * C_PER:(k + 1) * C_PER]                      # [125,11,1024]
        pad = np.zeros((NG * CPG, M, D), bf)
        pad[:C_PER] = sl
        grp = pad.reshape(NG, PG, D)
        full = np.zeros((NG, 128, D), bf)
        full[:, :PG] = grp
        rows = full.reshape(ROWS, D)
        nrows = 192 + 11 * 128 + 48
        membf = np.empty((nrows, D), bf)
        membf[:64] = img_b              # batch 0: img + g0
        membf[64:192] = rows[:128]
        membf[192:320] = maskem         # batch 1: mask/em
        membf[320:320 + 10 * 128] = rows[128:11 * 128]  # g1..g10
        membf[320 + 10 * 128:] = rows[11 * 128:11 * 128 + 48]  # g11 short
        in_maps.append({"membf": membf})
    return in_maps


def _gather(results):
    logits = np.empty((B, C), np.float32)
    for k in range(N_CORES):
        o = results[k]["out"].reshape(16, NG, 64)[:CPG]            # [11, 12, 64]
        o = o.transpose(1, 0, 2).reshape(NG * CPG, 64)[:C_PER]     # [125, 64]
        logits[:, k * C_PER:(k + 1) * C_PER] = o.T
    return logits


def kernel(img_features, memorized_image_feat):
    from concourse.bass_utils import run_bass_kernel_spmd

    nc = _get_nc()
    in_maps = _prep_inputs(img_features, memorized_image_feat)
    res = run_bass_kernel_spmd(nc, in_maps, core_ids=list(range(N_CORES)))
    return _gather(res.results)



# revision 2
# speedup vs baseline: 1.0849x; 1.0849x over previous
"""DualMem retrieval kernel for Trainium2 (8 NeuronCores, Bass/Tile).

Math (per reference):
    sim[b,c,m]  = <img[b], mem[c,m]>
    w           = exp(-beta * (1 - sim))
    adapt[b,c]  = sum_m mem[c,m] * w[b,c,m]
    logits[b,c] = 100 * <img[b], adapt[b,c] / ||adapt[b,c]||>

Key algebraic reduction (avoids materializing adapt [B,C,D]):
    numer[b,c]  = <img[b], adapt[b,c]> = sum_m w[b,c,m] * sim[b,c,m]
    denom[b,c]  = ||adapt[b,c]||^2     = w^T G_c w,  G_c = mem_c @ mem_c^T
    logits      = 100 * numer / sqrt(denom)

Sharding: classes C=1000 split 125 per core across 8 cores.

Precision/layout strategy (v2, vs the transpose-DMA bf16 baseline):
  - mem is shipped as float8 e3m4 scaled by 32 (1 byte/elem; values land in
    e3m4's normal range).  Host pre-transposes to [d%128, d//128, cm] so all
    input DMAs are plain contiguous copies (no xbar transpose).  img stays
    bf16; the PE accepts mixed e3m4(weights) x bf16(moving) matmuls.
    Measured end-to-end rel err ~1.05% (gate 2e-2).
  - The x32 scale cancels exactly: numer_psum = 32*numer and denom_psum =
    1024*denom, and 100*numer_psum/sqrt(denom_psum) = 100*numer/sqrt(denom).
    Only the exp gets scale beta/32.
  - Groups of 11 classes x 11 memories = 121 used partitions (+7 zero pad
    cols stored in the blob so every group is a uniform 128 cols).
  - Blocks of [4,4,3,1] groups share PSUM banks as in the baseline:
      su bank [128, gn*128]: group k cols 128k+0:64 = sim, 64:128 = u
      G bank  [128, gn*128]: Gram
      nd bank [16, gn*128] per block: [numer | denom] per group
  - Per-block finals use 100/sqrt(d) = exp(-0.5*ln(d) + ln(100)); Ln and Exp
    share one ACT table (pinned), loaded once.
  - Junk matmuls warm the PE p-state during the DMA startup window.
"""

import sys

sys.path.insert(0, "/opt/trn_rl_repo")

import ml_dtypes
import numpy as np

B, C, M, D = 64, 1000, 11, 1024
BETA = 5.5
SCALE = 32.0
N_CORES = 8
C_PER = C // N_CORES          # 125 classes per core
CPG = 11                      # classes per group
NG = 12                       # groups per core (132 class slots >= 125)
PG = CPG * M                  # 121 used partitions per group
DCH = D // 128                # 8 d-chunks

# blob byte layout (per partition): img bf16 | m1 bf16 | em bf16 | 12 groups
IMG_B = 2 * DCH * B           # 1024
M1_B = 2 * 128                # 256
EM_B = 2 * 16                 # 32
GOFF = IMG_B + M1_B + EM_B    # 2336
GW = [128] * (NG - 1) + [48]  # group col widths (incl zero pad cols)
GB = [DCH * w for w in GW]    # bytes per group per partition
BLOB_B = GOFF + sum(GB)       # 13984

# DMA chunk boundaries (byte cols): consts+g0+g1 | g2,g3 | g4-7 | g8-10 | g11
_g = [GOFF + sum(GB[:i]) for i in range(NG + 1)]
DMA_CUTS = [0, _g[2], _g[4], _g[8], _g[11], _g[12]]

BLKS = [(0, 4), (4, 4), (8, 3), (11, 1)]
N_JUNK = 8

_cache = {}


def _build():
    import concourse.mybir as mybir
    import concourse.tile as tile
    from concourse import bacc

    # Pin every activation to the one ACT table that holds BOTH Exp and Ln
    # (indices must be preserved -- empty the other sets instead of dropping
    # them) so the function table is loaded once and never swapped.
    if not getattr(bacc, "_act_tables_pinned", False):
        real = bacc.get_activation_tables

        def pinned(arch):
            return {k: (v if k == "natural_log_exp_and_others" else set())
                    for k, v in real(arch).items()}
        bacc.get_activation_tables = pinned
        bacc._act_tables_pinned = True

    f32 = mybir.dt.float32
    bf16 = mybir.dt.bfloat16
    f8e3 = mybir.dt.float8e3

    nc = bacc.Bacc("TRN2", target_bir_lowering=False, debug=False,
                   num_devices=N_CORES)

    blob = nc.dram_tensor("blob", [128, BLOB_B], f8e3, kind="ExternalInput")
    out = nc.dram_tensor("out", [16, NG * 64], f32, kind="ExternalOutput")

    with tile.TileContext(nc) as tc:
        with (
            tc.tile_pool(name="const", bufs=1) as const,
            tc.tile_pool(name="sb", bufs=3) as sb,
            tc.tile_pool(name="ps_su", bufs=2, space="PSUM") as ps_su,
            tc.tile_pool(name="ps_g", bufs=2, space="PSUM") as ps_g,
            tc.tile_pool(name="ps_nd", bufs=1, space="PSUM") as ps_nd,
        ):
            # one SBUF tile per DMA chunk so dependency tracking is per-chunk
            cb = [const.tile([128, DMA_CUTS[i + 1] - DMA_CUTS[i]], f8e3,
                             name=f"cb{i}", tag=f"cb{i}")
                  for i in range(len(DMA_CUTS) - 1)]
            for i, t in enumerate(cb):
                nc.sync.dma_start(
                    t[:], blob.ap()[:, DMA_CUTS[i]:DMA_CUTS[i + 1]])

            # views into the chunks
            imgT = cb[0][:, 0:IMG_B].bitcast(bf16) \
                .rearrange("p (j b) -> p j b", j=DCH)          # [128,8,64]
            m1 = cb[0][:, IMG_B:IMG_B + M1_B].bitcast(bf16)    # [128,128]
            em_bf = cb[0][:, IMG_B + M1_B:GOFF].bitcast(bf16)  # [128,16]

            def gview(g):
                off = _g[g]
                for i in range(len(cb)):
                    if DMA_CUTS[i] <= off < DMA_CUTS[i + 1]:
                        o = off - DMA_CUTS[i]
                        return cb[i][:, o:o + GB[g]].rearrange(
                            "p (j w) -> p j w", j=DCH)
                raise AssertionError
            gv_ = [gview(g) for g in range(NG)]

            lg = const.tile([16, NG * 64], f32)
            bias_exp = const.tile([128, 1], f32)
            bias_eps = const.tile([16, 1], f32)
            bias_ln100 = const.tile([16, 1], f32)
            junk_w = const.tile([128, 16], bf16)
            junk_x = const.tile([128, 512], bf16)
            nc.vector.memset(junk_w[:], 0)
            nc.vector.memset(junk_x[:], 0)
            nc.vector.memset(bias_exp[:], -BETA)
            nc.vector.memset(bias_eps[:], 1e-30)
            nc.vector.memset(bias_ln100[:], float(np.log(100.0)))

            # nd tiles: [numer | denom] per block; 4 PSUM banks total
            nd_t = [ps_nd.tile([16, gn * 128], f32, name=f"nd{nb}")
                    for nb, (g0, gn) in enumerate(BLKS)]

            # PE warm-up: the HAM clock gate (and the cost model) halve the
            # PE clock until ~3us of sustained activity.  These junk matmuls
            # have no DMA dependency, so they run during the DMA startup
            # window and the real matmuls start at full speed.  They scribble
            # on nd_t[0], which is rewritten (start=True) later.
            for _ in range(N_JUNK):
                nc.tensor.matmul(nd_t[0][:, 0:512], junk_w[:], junk_x[:],
                                 start=True, stop=True,
                                 skip_group_check=True)

            def emit_sims(nb, g0, gn):
                su = ps_su.tile([128, gn * 128], f32, tag="su", name=f"su{nb}")
                gp = ps_g.tile([128, gn * 128], f32, tag="gp", name=f"gp{nb}")
                for k in range(gn):
                    g = g0 + k
                    gw = GW[g]
                    gvv = 48 if g == NG - 1 else PG  # valid Gram columns
                    for i in range(DCH):
                        blk = gv_[g][:, i, 0:gw]
                        nc.tensor.matmul(su[0:gw, k * 128:k * 128 + 64],
                                         blk, imgT[:, i, :],
                                         start=(i == 0), stop=(i == DCH - 1),
                                         skip_group_check=True)
                        # i==0 writes all gw cols so the masked read later
                        # never sees uninitialized PSUM; pad cols keep the
                        # i==0 partial and are masked out
                        gvi = gw if i == 0 else gvv
                        nc.tensor.matmul(gp[0:gw, k * 128:k * 128 + gvi],
                                         blk, blk[:, 0:gvi],
                                         start=(i == 0), stop=(i == DCH - 1),
                                         skip_group_check=True)
                return su, gp

            def emit_down(nb, gn, su, gp):
                gw = GW[BLKS[nb][0] + gn - 1] if nb == 3 else 128
                su = su[0:gw]
                gp = gp[0:gw]
                # w = exp((beta/SCALE)*sim_psum - beta) for the whole block
                su4 = su.rearrange("p (k t b) -> p k t b", k=gn, t=2)
                w4 = sb.tile([128, gn * 64], bf16, tag="w4",
                             name=f"w4_{nb}")[0:gw]
                w4r = w4.rearrange("p (k b) -> p k b", k=gn)
                nc.scalar.activation(w4r, su4[:, :, 0, :],
                                     mybir.ActivationFunctionType.Exp,
                                     bias=bias_exp[0:gw],
                                     scale=BETA / SCALE)

                # masked Gram -> SBUF (kills cross-class + pad entries)
                gm4 = sb.tile([128, gn * 128], bf16, tag="gm4",
                              name=f"gm4_{nb}")[0:gw]
                if nb == 3:
                    nc.vector.tensor_mul(gm4[:, 0:gw], gp[:, 0:gw],
                                         m1[0:gw, 0:gw])
                else:
                    gp4 = gp.rearrange("p (k j) -> p k j", k=gn)
                    m1b = m1.rearrange("p (u j) -> p u j", u=1) \
                        .to_broadcast((gw, gn, 128))
                    nc.vector.tensor_mul(
                        gm4.rearrange("p (k j) -> p k j", k=gn), gp4, m1b)

                # u_k = G_k^T @ w_k, placed next to sim_k in the same bank
                for k in range(gn):
                    nc.tensor.matmul(su[:, k * 128 + 64:(k + 1) * 128],
                                     gm4[:, k * 128:k * 128 + gw],
                                     w4[:, k * 64:(k + 1) * 64],
                                     start=True, stop=True,
                                     skip_group_check=True)

                # wsq = [w*sim | w*u], one fused mul with w broadcast over t
                wsq = sb.tile([128, gn * 128], bf16, tag="wsq",
                              name=f"wsq_{nb}")[0:gw]
                wq4 = wsq.rearrange("p (k t b) -> p k t b", k=gn, t=2)
                w4b = w4.rearrange("p (k u b) -> p k u b", k=gn, u=1) \
                    .to_broadcast((gw, gn, 2, 64))
                nc.vector.tensor_mul(wq4, su4, w4b)

                # nd[c, :] = [numer | denom] per class for the whole block
                nc.tensor.matmul(nd_t[nb][:], em_bf[0:gw], wsq, start=True,
                                 stop=True, skip_group_check=True)

            def emit_finals(nb, gn):
                # 100/sqrt(denom) = exp(-0.5*ln(denom) + ln(100)); Ln and Exp
                # live in the same ACT table, so no table swap ever
                g0 = BLKS[nb][0]
                nd3 = nd_t[nb][:].rearrange("p (g t b) -> p g t b", g=gn, t=2)
                s_h = sb.tile([16, gn * 64], f32, tag=f"s{nb}",
                              name=f"s_{nb}")
                nc.scalar.activation(s_h[:], nd3[:, :, 1, :],
                                     mybir.ActivationFunctionType.Ln,
                                     bias=bias_eps[:], scale=1.0)
                r_h = sb.tile([16, gn * 64], f32, tag=f"r{nb}",
                              name=f"r_{nb}")
                nc.scalar.activation(r_h[:], s_h[:],
                                     mybir.ActivationFunctionType.Exp,
                                     bias=bias_ln100[:], scale=-0.5)
                o0 = g0 * 64
                nc.vector.tensor_mul(lg[:, o0:o0 + gn * 64], nd3[:, :, 0, :],
                                     r_h[:])
                nc.sync.dma_start(out.ap()[:, o0:o0 + gn * 64],
                                  lg[:, o0:o0 + gn * 64])

            # Emit each block's sim/G matmuls BEFORE the previous block's
            # downstream ops so the PE queue never head-blocks on exp/mask,
            # and finals trail one more block so Act/DVE stay off the PE
            # critical path.
            pend = []
            done = []
            for nb, (g0, gn) in enumerate(BLKS):
                su, gp = emit_sims(nb, g0, gn)
                pend.append((nb, gn, su, gp))
                if len(pend) >= 2:
                    args = pend.pop(0)
                    emit_down(*args)
                    done.append((args[0], args[1]))
                if len(done) >= 2:
                    emit_finals(*done.pop(0))
            for args in pend:
                emit_down(*args)
                done.append((args[0], args[1]))
            for args in done:
                emit_finals(*args)

    nc.compile()
    return nc


def _get_nc():
    if "nc" not in _cache:
        _cache["nc"] = _build()
    return _cache["nc"]


def _prep_inputs(img_features, memorized_image_feat):
    """Host-side formatting: dtype casts, x32 scale, transpose, group blob."""
    bf = ml_dtypes.bfloat16
    f8 = ml_dtypes.float8_e3m4
    img_b = np.ascontiguousarray(img_features.astype(bf))       # [64, 1024]
    mem8 = (memorized_image_feat * SCALE).astype(f8)            # [1000,11,1024]

    # imgT bytes: it[p, j, b] = img[b, j*128+p]
    it = img_b.T.reshape(DCH, 128, B).transpose(1, 0, 2)        # [128, 8, 64]
    it_bytes = np.ascontiguousarray(it).view(np.uint8).reshape(128, IMG_B)

    m1 = np.zeros((128, 128), np.float32)
    for c in range(CPG):
        m1[c * M:(c + 1) * M, c * M:(c + 1) * M] = 1.0
    m1_bytes = m1.astype(bf).view(np.uint8).reshape(128, M1_B)
    em = np.zeros((128, 16), np.float32)
    for c in range(CPG):
        em[c * M:(c + 1) * M, c] = 1.0
    em_bytes = em.astype(bf).view(np.uint8).reshape(128, EM_B)

    in_maps = []
    for k in range(N_CORES):
        sl = mem8[k * C_PER:(k + 1) * C_PER].reshape(C_PER * M, D)
        blob = np.zeros((128, BLOB_B), np.uint8)
        blob[:, 0:IMG_B] = it_bytes
        blob[:, IMG_B:IMG_B + M1_B] = m1_bytes
        blob[:, IMG_B + M1_B:GOFF] = em_bytes
        for g in range(NG):
            r0, gw = g * PG, GW[g]
            rows = np.zeros((gw, D), f8)
            n = min(PG, C_PER * M - r0)
            rows[:n] = sl[r0:r0 + n]
            # memT[p, j, w] = rows[w, j*128+p]
            mt = rows.T.reshape(DCH, 128, gw).transpose(1, 0, 2)  # [128,8,gw]
            blob[:, _g[g]:_g[g + 1]] = np.ascontiguousarray(mt) \
                .view(np.uint8).reshape(128, GB[g])
        in_maps.append({"blob": blob.view(f8)})
    return in_maps


def _gather(results):
    logits = np.empty((B, C), np.float32)
    for k in range(N_CORES):
        o = results[k]["out"].reshape(16, NG, 64)[:CPG]         # [11, 12, 64]
        o = o.transpose(1, 0, 2).reshape(NG * CPG, 64)[:C_PER]  # [125, 64]
        logits[:, k * C_PER:(k + 1) * C_PER] = o.T
    return logits


def kernel(img_features, memorized_image_feat):
    from concourse.bass_utils import run_bass_kernel_spmd

    nc = _get_nc()
    in_maps = _prep_inputs(img_features, memorized_image_feat)
    res = run_bass_kernel_spmd(nc, in_maps, core_ids=list(range(N_CORES)))
    return _gather(res.results)


# revision 3
# speedup vs baseline: 1.1003x; 1.0143x over previous
"""DualMem retrieval kernel for Trainium2 (8 NeuronCores, Bass/Tile).

Math (per reference):
    sim[b,c,m]  = <img[b], mem[c,m]>
    w           = exp(-beta * (1 - sim))
    adapt[b,c]  = sum_m mem[c,m] * w[b,c,m]
    logits[b,c] = 100 * <img[b], adapt[b,c] / ||adapt[b,c]||>

Key algebraic reduction (avoids materializing adapt [B,C,D]):
    numer[b,c]  = <img[b], adapt[b,c]> = sum_m w[b,c,m] * sim[b,c,m]
    denom[b,c]  = ||adapt[b,c]||^2     = w^T G_c w,  G_c = mem_c @ mem_c^T
    logits      = 100 * numer / sqrt(denom)

Sharding: classes C=1000 split 125 per core across 8 cores.

Precision/layout strategy:
  - mem is shipped as float8 e3m4 scaled by 32 (1 byte/elem; values land in
    e3m4's normal range).  Host pre-transposes to [d%128, d//128, cm] so all
    input DMAs are plain contiguous copies (no xbar transpose).  img stays
    bf16; the PE accepts mixed e3m4(weights) x bf16(moving) matmuls.
    Measured end-to-end rel err ~1.05% (gate 2e-2).
  - The x32 scale cancels exactly: numer_psum = 32*numer and denom_psum =
    1024*denom, and 100*numer_psum/sqrt(denom_psum) = 100*numer/sqrt(denom).
    Only the exp gets scale beta/32.
  - Groups of 11 classes x 11 memories = 121 used partitions (+7 zero pad
    cols stored in the blob so every group is a uniform 128 cols).
  - Blocks of [4,4,3,1] groups share PSUM banks:
      su bank [128, gn*128]: group k cols 128k+0:64 = sim, 64:128 = u
      G bank  [128, gn*128]: Gram
      nd bank [16, gn*128] per block: [all numer | all denom]
  - w*sim is computed right after the exp (independent of u) and w*u after
    the u matmul, with separate numer/denom nd matmuls, so only the wu->nd_d
    link sits on each block's critical tail.
  - Finals (100/sqrt(d) = exp(-0.5*ln(d)+ln(100)); Ln+Exp share one pinned
    ACT table) are merged for blocks A+B and C+D and emitted after ALL exps
    so the strictly in-order Activation queue never head-blocks an exp
    behind a final that waits on a late nd.
  - Junk matmuls (128-col, fine-grained) warm the PE p-state during the DMA
    startup window so real matmuls run at full clock from the start.
"""

import sys

sys.path.insert(0, "/opt/trn_rl_repo")

import ml_dtypes
import numpy as np

B, C, M, D = 64, 1000, 11, 1024
BETA = 5.5
SCALE = 32.0
N_CORES = 8
C_PER = C // N_CORES          # 125 classes per core
CPG = 11                      # classes per group
NG = 12                       # groups per core (132 class slots >= 125)
PG = CPG * M                  # 121 used partitions per group
DCH = D // 128                # 8 d-chunks

IMG_B = 2 * DCH * B           # 1024 bytes/partition of bf16 imgT
M1_B = 2 * 128                # 256
EM_B = 2 * 16                 # 32
GW = [128] * (NG - 1) + [48]  # group col widths (incl zero pad cols)
GB = [DCH * w for w in GW]    # bytes per group per partition

# blob layout (per-partition byte cols), ordered to match the DMA chunks:
#   d0: img g0 g1 | d1: g2 g3 m1 em | d2: g4-g7 | d3: g8-g10 | d4: g11
_off = {}
_cur = 0
def _seg(name, nbytes):
    global _cur
    _off[name] = _cur
    _cur += nbytes
_seg("img", IMG_B)
_seg("g0", GB[0]); _seg("g1", GB[1]); _seg("g2", GB[2]); _seg("g3", GB[3])
_seg("m1", M1_B); _seg("em", EM_B)
for g in range(4, NG):
    _seg(f"g{g}", GB[g])
BLOB_B = _cur
DMA_CUTS = [0, _off["g2"], _off["g4"], _off["g8"], _off["g11"],
            _off["g11"] + GB[11]]

BLKS = [(0, 4), (4, 4), (8, 3), (11, 1)]
N_JUNK = 30

_cache = {}


def _build():
    import concourse.mybir as mybir
    import concourse.tile as tile
    from concourse import bacc

    # Pin every activation to the one ACT table that holds BOTH Exp and Ln
    # (indices must be preserved -- empty the other sets instead of dropping
    # them) so the function table is loaded once and never swapped.
    if not getattr(bacc, "_act_tables_pinned", False):
        real = bacc.get_activation_tables

        def pinned(arch):
            return {k: (v if k == "natural_log_exp_and_others" else set())
                    for k, v in real(arch).items()}
        bacc.get_activation_tables = pinned
        bacc._act_tables_pinned = True

    f32 = mybir.dt.float32
    bf16 = mybir.dt.bfloat16
    f8e3 = mybir.dt.float8e3

    nc = bacc.Bacc("TRN2", target_bir_lowering=False, debug=False,
                   num_devices=N_CORES)

    blob = nc.dram_tensor("blob", [128, BLOB_B], f8e3, kind="ExternalInput")
    out = nc.dram_tensor("out", [16, NG * 64], f32, kind="ExternalOutput")

    with tile.TileContext(nc) as tc:
        with (
            tc.tile_pool(name="const", bufs=1) as const,
            tc.tile_pool(name="sb", bufs=3) as sb,
            tc.tile_pool(name="ps_su", bufs=2, space="PSUM") as ps_su,
            tc.tile_pool(name="ps_g", bufs=2, space="PSUM") as ps_g,
            tc.tile_pool(name="ps_nd", bufs=1, space="PSUM") as ps_nd,
        ):
            # one SBUF tile per DMA chunk so dependency tracking is per-chunk
            cb = [const.tile([128, DMA_CUTS[i + 1] - DMA_CUTS[i]], f8e3,
                             name=f"cb{i}", tag=f"cb{i}")
                  for i in range(len(DMA_CUTS) - 1)]
            for i, t in enumerate(cb):
                nc.sync.dma_start(
                    t[:], blob.ap()[:, DMA_CUTS[i]:DMA_CUTS[i + 1]])

            def view(name, nbytes):
                off = _off[name]
                for i in range(len(cb)):
                    if DMA_CUTS[i] <= off < DMA_CUTS[i + 1]:
                        o = off - DMA_CUTS[i]
                        return cb[i][:, o:o + nbytes]
                raise AssertionError

            imgT = view("img", IMG_B).bitcast(bf16) \
                .rearrange("p (j b) -> p j b", j=DCH)          # [128,8,64]
            m1 = view("m1", M1_B).bitcast(bf16)                # [128,128]
            em_bf = view("em", EM_B).bitcast(bf16)             # [128,16]
            gv_ = [view(f"g{g}", GB[g]).rearrange("p (j w) -> p j w", j=DCH)
                   for g in range(NG)]

            lg = const.tile([16, NG * 64], f32)
            bias_exp = const.tile([128, 1], f32)
            bias_eps = const.tile([16, 1], f32)
            bias_ln100 = const.tile([16, 1], f32)
            junk_w = const.tile([128, 16], bf16)
            junk_x = const.tile([128, 128], bf16)
            # junk feedstock memset on Pool so the PE warm-up starts early;
            # bias memsets on DVE (needed much later)
            nc.gpsimd.memset(junk_w[:], 0)
            nc.gpsimd.memset(junk_x[:], 0)
            nc.vector.memset(bias_exp[:], -BETA)
            nc.vector.memset(bias_eps[:], 1e-30)
            nc.vector.memset(bias_ln100[:], float(np.log(100.0)))

            # nd tiles: [all numer | all denom] per block; 4 PSUM banks
            nd_t = [ps_nd.tile([16, gn * 128], f32, name=f"nd{nb}")
                    for nb, (g0, gn) in enumerate(BLKS)]

            # PE warm-up: the HAM clock gate (and the cost model) halve the
            # PE clock until ~3us of sustained activity.  Fine-grained junk
            # matmuls (no DMA dependency) run during the DMA startup window;
            # they scribble on nd_t[0], rewritten (start=True) later.
            for _ in range(N_JUNK):
                nc.tensor.matmul(nd_t[0][:, 0:128], junk_w[:], junk_x[:],
                                 start=True, stop=True,
                                 skip_group_check=True)

            def emit_sims(nb, g0, gn):
                su = ps_su.tile([128, gn * 128], f32, tag="su", name=f"su{nb}")
                gp = ps_g.tile([128, gn * 128], f32, tag="gp", name=f"gp{nb}")
                for k in range(gn):
                    g = g0 + k
                    gw = GW[g]
                    gvv = 48 if g == NG - 1 else PG  # valid Gram columns
                    for i in range(DCH):
                        blk = gv_[g][:, i, 0:gw]
                        nc.tensor.matmul(su[0:gw, k * 128:k * 128 + 64],
                                         blk, imgT[:, i, :],
                                         start=(i == 0), stop=(i == DCH - 1),
                                         skip_group_check=True)
                        # i==0 writes all gw cols so the masked read later
                        # never sees uninitialized PSUM; pad cols keep the
                        # i==0 partial and are masked out
                        gvi = gw if i == 0 else gvv
                        nc.tensor.matmul(gp[0:gw, k * 128:k * 128 + gvi],
                                         blk, blk[:, 0:gvi],
                                         start=(i == 0), stop=(i == DCH - 1),
                                         skip_group_check=True)
                return su, gp

            def emit_down(nb, gn, su, gp):
                gw = 48 if nb == len(BLKS) - 1 else 128
                su = su[0:gw]
                gp = gp[0:gw]
                nd = nd_t[nb]
                # w = exp((beta/SCALE)*sim_psum - beta) for the whole block
                su4 = su.rearrange("p (k t b) -> p k t b", k=gn, t=2)
                w4 = sb.tile([128, gn * 64], bf16, tag="w4",
                             name=f"w4_{nb}")[0:gw]
                w4r = w4.rearrange("p (k b) -> p k b", k=gn)
                nc.scalar.activation(w4r, su4[:, :, 0, :],
                                     mybir.ActivationFunctionType.Exp,
                                     bias=bias_exp[0:gw],
                                     scale=BETA / SCALE)

                # masked Gram -> SBUF (kills cross-class + pad entries)
                gm4 = sb.tile([128, gn * 128], bf16, tag="gm4",
                              name=f"gm4_{nb}")[0:gw]
                if gn == 1:
                    nc.vector.tensor_mul(gm4[:, 0:gw], gp[:, 0:gw],
                                         m1[0:gw, 0:gw])
                else:
                    gp4 = gp.rearrange("p (k j) -> p k j", k=gn)
                    m1b = m1.rearrange("p (u j) -> p u j", u=1) \
                        .to_broadcast((gw, gn, 128))
                    nc.vector.tensor_mul(
                        gm4.rearrange("p (k j) -> p k j", k=gn), gp4, m1b)

                # w*sim immediately (independent of u) -> numer matmul early
                ws = sb.tile([128, gn * 64], bf16, tag="ws",
                             name=f"ws_{nb}")[0:gw]
                ws3 = ws.rearrange("p (k b) -> p k b", k=gn)
                nc.vector.tensor_mul(ws3, su4[:, :, 0, :], w4r)
                nc.tensor.matmul(nd[:, 0:gn * 64], em_bf[0:gw], ws,
                                 start=True, stop=True, skip_group_check=True)

                # u_k = G_k^T @ w_k, placed next to sim_k in the same bank
                for k in range(gn):
                    nc.tensor.matmul(su[:, k * 128 + 64:(k + 1) * 128],
                                     gm4[:, k * 128:k * 128 + gw],
                                     w4[:, k * 64:(k + 1) * 64],
                                     start=True, stop=True,
                                     skip_group_check=True)

                # w*u -> denom matmul
                wu = sb.tile([128, gn * 64], bf16, tag="wu",
                             name=f"wu_{nb}")[0:gw]
                wu3 = wu.rearrange("p (k b) -> p k b", k=gn)
                nc.vector.tensor_mul(wu3, su4[:, :, 1, :], w4r)
                nc.tensor.matmul(nd[:, gn * 64:gn * 128], em_bf[0:gw], wu,
                                 start=True, stop=True, skip_group_check=True)

            def emit_finals(blocks):
                # 100/sqrt(denom) = exp(-0.5*ln(denom) + ln(100)); Ln and Exp
                # live in the same ACT table, so no table swap ever.
                # One ln/exp/mul over the concatenated denom cols of `blocks`.
                tag = "f" + "".join(str(nb) for nb in blocks)
                tot = sum(BLKS[nb][1] for nb in blocks) * 64
                s_h = sb.tile([16, tot], f32, tag=f"s{tag}", name=f"s_{tag}")
                o = 0
                for nb in blocks:
                    gn = BLKS[nb][1]
                    nc.scalar.activation(
                        s_h[:, o:o + gn * 64],
                        nd_t[nb][:, gn * 64:gn * 128],
                        mybir.ActivationFunctionType.Ln,
                        bias=bias_eps[:], scale=1.0)
                    o += gn * 64
                r_h = sb.tile([16, tot], f32, tag=f"r{tag}", name=f"r_{tag}")
                nc.scalar.activation(r_h[:], s_h[:],
                                     mybir.ActivationFunctionType.Exp,
                                     bias=bias_ln100[:], scale=-0.5)
                o = 0
                for nb in blocks:
                    g0, gn = BLKS[nb]
                    nc.vector.tensor_mul(lg[:, g0 * 64:(g0 + gn) * 64],
                                         nd_t[nb][:, 0:gn * 64],
                                         r_h[:, o:o + gn * 64])
                    o += gn * 64
                lo = BLKS[blocks[0]][0] * 64
                hi = (BLKS[blocks[-1]][0] + BLKS[blocks[-1]][1]) * 64
                nc.sync.dma_start(out.ap()[:, lo:hi], lg[:, lo:hi])

            # Emit each block's sim/G matmuls BEFORE the previous block's
            # downstream ops so the PE queue never head-blocks on exp/mask;
            # finals go last so the in-order Act queue runs all exps first.
            pend = []
            for nb, (g0, gn) in enumerate(BLKS):
                su, gp = emit_sims(nb, g0, gn)
                pend.append((nb, gn, su, gp))
                if len(pend) >= 2:
                    emit_down(*pend.pop(0))
            for args in pend:
                emit_down(*args)
            emit_finals([0, 1])
            emit_finals([2, 3])

    nc.compile()
    return nc


def _get_nc():
    if "nc" not in _cache:
        _cache["nc"] = _build()
    return _cache["nc"]


def _prep_inputs(img_features, memorized_image_feat):
    """Host-side formatting: dtype casts, x32 scale, transpose, group blob."""
    bf = ml_dtypes.bfloat16
    f8 = ml_dtypes.float8_e3m4
    img_b = np.ascontiguousarray(img_features.astype(bf))       # [64, 1024]
    mem8 = (memorized_image_feat * SCALE).astype(f8)            # [1000,11,1024]

    # imgT bytes: it[p, j, b] = img[b, j*128+p]
    it = img_b.T.reshape(DCH, 128, B).transpose(1, 0, 2)        # [128, 8, 64]
    it_bytes = np.ascontiguousarray(it).view(np.uint8).reshape(128, IMG_B)

    m1 = np.zeros((128, 128), np.float32)
    for c in range(CPG):
        m1[c * M:(c + 1) * M, c * M:(c + 1) * M] = 1.0
    m1_bytes = m1.astype(bf).view(np.uint8).reshape(128, M1_B)
    em = np.zeros((128, 16), np.float32)
    for c in range(CPG):
        em[c * M:(c + 1) * M, c] = 1.0
    em_bytes = em.astype(bf).view(np.uint8).reshape(128, EM_B)

    in_maps = []
    for k in range(N_CORES):
        sl = mem8[k * C_PER:(k + 1) * C_PER].reshape(C_PER * M, D)
        blob = np.zeros((128, BLOB_B), np.uint8)
        blob[:, _off["img"]:_off["img"] + IMG_B] = it_bytes
        blob[:, _off["m1"]:_off["m1"] + M1_B] = m1_bytes
        blob[:, _off["em"]:_off["em"] + EM_B] = em_bytes
        for g in range(NG):
            r0, gw = g * PG, GW[g]
            rows = np.zeros((gw, D), f8)
            n = min(PG, C_PER * M - r0)
            rows[:n] = sl[r0:r0 + n]
            # memT[p, j, w] = rows[w, j*128+p]
            mt = rows.T.reshape(DCH, 128, gw).transpose(1, 0, 2)  # [128,8,gw]
            blob[:, _off[f"g{g}"]:_off[f"g{g}"] + GB[g]] = \
                np.ascontiguousarray(mt).view(np.uint8).reshape(128, GB[g])
        in_maps.append({"blob": blob.view(f8)})
    return in_maps


def _gather(results):
    logits = np.empty((B, C), np.float32)
    for k in range(N_CORES):
        o = results[k]["out"].reshape(16, NG, 64)[:CPG]         # [11, 12, 64]
        o = o.transpose(1, 0, 2).reshape(NG * CPG, 64)[:C_PER]  # [125, 64]
        logits[:, k * C_PER:(k + 1) * C_PER] = o.T
    return logits


def kernel(img_features, memorized_image_feat):
    from concourse.bass_utils import run_bass_kernel_spmd

    nc = _get_nc()
    in_maps = _prep_inputs(img_features, memorized_image_feat)
    res = run_bass_kernel_spmd(nc, in_maps, core_ids=list(range(N_CORES)))
    return _gather(res.results)
